# revision 36
# baseline (speedup 1.0000x reference)
"""Multi-head attention (B=8, N=1024, DIM=1152, H=16, hd=72) on 8 TRN2 cores.

Sharding: pure data parallelism -- core i computes batch element i, weights
are replicated. No collectives.

Wavefront schedule (per core): QK projection m-tiles are emitted in
(Q_j, K_j) PAIRS so head j's attention chain (repack -> S -> exp -> AV ->
normalize) runs one round behind the projection matmuls that produce its
rows.  This spreads the ~120us of ScalarE exp work over the whole kernel
instead of concentrating it in an attention phase where it outpaces the
PE (3.8us exp vs 3.5us matmul per head-block), and removes the cold-start
serialization (w/x DMAs ordered minimal-dependency-first across queues).

Perf notes from trace analysis (HW ~284-286us, PE-saturated):
  - PE stream floor is ~247.5us @2.4GHz; prologue+teardown barriers are
    ~16us fixed; startup x-feed is chip-HBM-bound (~6us idle).
  - S operands are padded to 96 contraction partitions with zero rows
    72..95 (<96-partition matmuls measured 2 cyc/row in isolation).
  - wproj k-tiles 0..2 are prefetched during round 7 into wp_early so
    the projection phase never stalls on weight DMAs; proj rotates 4
    PSUM banks (s/o pools freed first).
  - wv is loaded in per-vb column chunks so round-r v_chains gate only
    on their own chunk; wqk prefetches go on sync/gpsimd (scalar ring
    carries the slow strided wv transfers).
  - fp8 DoubleRow was measured at 1.0 cyc/output-row (2x contraction
    only); e4m3 rounding (~2.4%/operand) blows the 2e-2 budget on any
    single matmul, so everything stays bf16.

Numerics / layout notes (inherited from the phase-split version):
  - x arrives bf16 (host cast); x^T built by host relayout.
  - S^T = K_h @ Q_h^T puts softmax's k-reduction on PSUM partitions; the
    denominator is recovered free via a ones column appended to V.
  - exp on ScalarE with the 1/sqrt(hd) scale folded in; no max subtraction
    (scores are ~N(0,1)).
  - Normalization: denominator row stream_shuffled to quadrant 0,
    reciprocal_approx_fast, shuffled back, fused DVE multiply into bf16
    O^T, DMA-repacked into a compact [128, 9, N] stack for the projection.
"""

import sys

sys.path.insert(0, "/opt/trn_rl_repo")

import numpy as np
import ml_dtypes

B, N, DIM, HEADS = 8, 1024, 1152, 16
HD = DIM // HEADS  # 72
NCORES = 8
QKDIM = 2 * DIM  # 2304 (q and k outdims concatenated)
N_MT_QK = QKDIM // 128  # 18 m-tiles for Q,K
N_PAIR = 9  # (Q_j, K_j) m-tile pairs
N_KT = DIM // 128  # 9 contraction tiles
N_TT = N // 128  # 8 token tiles
QB = 512  # q block (moving dim) for S^T / qkv
N_QB = N // QB  # 2
VB = 288  # v block = 4 heads
N_VB = DIM // VB  # 4
EB = 384  # proj output block
N_EB = DIM // EB  # 3

# heads whose Q/K rows are fully covered once m-tile pairs 0..j are done
HEADS_BY_PAIR = [[], [0], [1, 2], [3, 4], [5, 6], [7], [8, 9], [10, 11],
                 [12, 13], [14, 15]]  # index = round r; heads from pair r-1
# (vb, token tiles) of V-projection chains per round; vb3's last two chains
# sit in round 8 so its exp burst has PE cover (AV h12/h13 needs vb3 done)
V_BY_ROUND = {0: (0, [0, 1, 2, 3]), 1: (0, [4, 5, 6, 7]),
              2: (1, [0, 1, 2, 3]), 3: (1, [4, 5, 6, 7]),
              4: (2, [0, 1, 2, 3]), 5: (2, [4, 5, 6, 7]),
              6: (3, [0, 1, 2, 3]), 7: (3, [4, 5]), 8: (3, [6, 7])}

_CACHE = {}


def _head_pieces(h):
    """Pieces covering rows [72h, 72h+72) of a 128-row-tiled stack, as
    (mtile, src_lo, src_hi, dst_lo)."""
    r0 = HD * h
    mt, p0 = divmod(r0, 128)
    ln = min(HD, 128 - p0)
    pieces = [(mt, p0, p0 + ln, 0)]
    if ln < HD:
        pieces.append((mt + 1, 0, HD - ln, ln))
    return pieces


def _build(debug_taps=False):
    import concourse.tile as tile
    from concourse import bacc, mybir

    f32 = mybir.dt.float32
    bf16 = mybir.dt.bfloat16
    Exp = mybir.ActivationFunctionType.Exp

    nc = bacc.Bacc("TRN2", target_bir_lowering=False, debug=False,
                   num_devices=NCORES)

    x_d = nc.dram_tensor("x", [128, N_KT, N], bf16,
                         kind="ExternalInput").ap()  # x^T, host-relayouted
    wqk_d = nc.dram_tensor("wqk", [N_MT_QK, 128, N_KT, 128], bf16,
                           kind="ExternalInput").ap()
    wv_d = nc.dram_tensor("wv", [DIM, DIM], bf16, kind="ExternalInput").ap()
    bqk_d = nc.dram_tensor("bqk", [128, N_MT_QK], f32,
                           kind="ExternalInput").ap()
    bv_d = nc.dram_tensor("bv", [1, DIM], f32, kind="ExternalInput").ap()
    wproj_d = nc.dram_tensor("wproj", [128, N_KT, DIM], bf16,
                             kind="ExternalInput").ap()
    bproj_d = nc.dram_tensor("bproj", [1, DIM], f32,
                             kind="ExternalInput").ap()
    out_d = nc.dram_tensor("out", [N, DIM], f32, kind="ExternalOutput").ap()
    if debug_taps:
        qkt_d = nc.dram_tensor("dbg_qkt", [128, N_MT_QK, N], f32,
                               kind="ExternalOutput").ap()
        vpad_d = nc.dram_tensor("dbg_vpad", [128, N_TT, HEADS, HD + 1], f32,
                                kind="ExternalOutput").ap()
        ot_d = nc.dram_tensor("dbg_ot", [128, N_KT, N], f32,
                              kind="ExternalOutput").ap()

    scale = float(HD) ** -0.5

    with tile.TileContext(nc) as tc:
        with tc.tile_pool(name="consts", bufs=1) as consts, \
             tc.tile_pool(name="persist", bufs=1) as persist:
            # ---- persistent activations ----
            qkt = persist.tile([128, N_MT_QK, N], bf16)   # Q^T,K^T stacked
            vpad = persist.tile([128, N_TT, HEADS, HD + 1], bf16)
            ot = persist.tile([128, N_KT, N], bf16)       # O^T compact stack

            bqk_sb = consts.tile([128, N_MT_QK], f32)
            bv_sb = consts.tile([128, DIM], f32)
            bproj_sb = consts.tile([128, DIM], f32)

            # pools that live for the whole schedule
            pad_stack = tc.tile_pool(name="qk_pad", bufs=1)
            pads = pad_stack.__enter__()
            es_stack = tc.tile_pool(name="es_pool", bufs=16)
            es_pool = es_stack.__enter__()
            r_stack = tc.tile_pool(name="r_pool", bufs=5)
            r_pool = r_stack.__enter__()
            wp_early_stack = tc.tile_pool(name="wp_early", bufs=1)
            wp_early_pool = wp_early_stack.__enter__()
            wp_early = wp_early_pool.tile([128, 3, DIM], bf16)
            mm_stack = tc.tile_pool(name="mm_ps", bufs=1, space="PSUM")
            mm_ps = mm_stack.__enter__()
            s_stack = tc.tile_pool(name="s_ps", bufs=2, space="PSUM")
            s_ps = s_stack.__enter__()
            o_stack = tc.tile_pool(name="o_ps", bufs=2, space="PSUM")
            o_ps = o_stack.__enter__()

            ident = list(range(32))

            def qk_chain(xt, w_t, m, qb):
                ps = mm_ps.tile([128, QB], f32, tag="mm", bufs=2)
                for kt in range(N_KT):
                    nc.tensor.matmul(
                        ps,
                        lhsT=w_t[:, kt, :],
                        rhs=xt[:, kt, qb * QB:(qb + 1) * QB],
                        start=(kt == 0), stop=(kt == N_KT - 1))
                nc.scalar.add(
                    qkt[:, m, qb * QB:(qb + 1) * QB], ps, bqk_sb[:, m:m + 1])

            def v_chain(xt, wv_sb, vb, tt):
                ps = mm_ps.tile([128, QB], f32, tag="mm", bufs=2)
                for kt in range(N_KT):
                    nc.tensor.matmul(
                        ps[:, 0:VB],
                        lhsT=xt[:, kt, tt * 128:(tt + 1) * 128],
                        rhs=wv_sb[:, kt, vb * VB:(vb + 1) * VB],
                        start=(kt == 0), stop=(kt == N_KT - 1))
                nc.vector.tensor_add(
                    vpad[:, tt, 4 * vb:4 * vb + 4, 0:HD],
                    ps[:, 0:VB].rearrange("p (g d) -> p g d", g=4),
                    bv_sb[:, vb * VB:(vb + 1) * VB].rearrange(
                        "p (g d) -> p g d", g=4))

            # Repack targets: persistent tiles rotated manually so the
            # zero rows 72..95 survive across rounds (matmuls with <96
            # contraction partitions run at 2 cycles/row on TRN2, so S
            # operands are padded to 96 with zeros; zeros on BOTH sides so
            # no stale-NaN x 0 = NaN).
            ktp_bufs = [pads.tile([128, N], bf16, tag=f"ktp{i}",
                                  name=f"ktp{i}") for i in range(4)]
            qtp_bufs = [pads.tile([128, N], bf16, tag=f"qtp{i}",
                                  name=f"qtp{i}") for i in range(4)]
            for i in range(4):
                # 32-aligned partition base; rows 64..71 are re-written by
                # every repack DMA, rows 72..95 stay zero forever.
                nc.vector.memset(ktp_bufs[i][64:96, :], 0.0)
                nc.vector.memset(qtp_bufs[i][64:96, :], 0.0)
            repack_ctr = [0]

            def issue_repacks(h):
                """SBUF->SBUF DMAs move head h's K^T/Q^T rows to partition 0
                (matmul operands must start at partition 0/32/64)."""
                pieces = _head_pieces(h)
                i = repack_ctr[0] % 4
                repack_ctr[0] += 1
                ktp = ktp_bufs[i]
                qtp = qtp_bufs[i]
                for (mt, lo, hi, dst) in pieces:
                    nc.sync.dma_start(ktp[dst:dst + hi - lo, :],
                                      qkt[lo:hi, 9 + mt, :])
                    nc.gpsimd.dma_start(qtp[dst:dst + hi - lo, :],
                                        qkt[lo:hi, mt, :])
                return ktp, qtp

            def s_block(h, qb, ktp, qtp):
                es_tiles = []
                for kp in range(N_TT // 2):
                    ps = s_ps.tile([128, 2, QB], f32, tag="s")
                    for j in range(2):
                        kt = 2 * kp + j
                        nc.tensor.matmul(
                            ps[:, j, :],
                            lhsT=ktp[0:96, kt * 128:(kt + 1) * 128],
                            rhs=qtp[0:96, qb * QB:(qb + 1) * QB],
                            start=True, stop=True)
                    es = es_pool.tile([128, 2, QB], bf16, tag="e")
                    nc.scalar.activation(es, ps, func=Exp, scale=scale)
                    es_tiles.append(es)
                return es_tiles

            def av_block(h, qb, es_tiles, nrm_q):
                pieces = _head_pieces(h)
                ops = o_ps.tile([128, QB], f32, tag="o")
                for kt in range(N_TT):
                    nc.tensor.matmul(
                        ops[0:HD + 1, :],
                        lhsT=vpad[:, kt, h, :],
                        rhs=es_tiles[kt // 2][:, kt % 2, :],
                        start=(kt == 0), stop=(kt == N_TT - 1))
                # denominator (psum row 72) -> reciprocal broadcast rows 0..71
                rt = r_pool.tile([96, 2 * QB], f32, tag="r")
                nc.vector.stream_shuffle(
                    rt[0:32, 0:QB], ops[64:96, :], mask=[8] * 32)
                nc.vector.reciprocal_approx_fast(
                    rt[0:32, QB:2 * QB], rt[0:32, 0:QB])
                nc.vector.stream_shuffle(
                    rt[32:64, QB:2 * QB], rt[0:32, QB:2 * QB], mask=ident)
                nc.vector.stream_shuffle(
                    rt[64:96, QB:2 * QB], rt[0:32, QB:2 * QB], mask=ident)
                otp = pads.tile([128, QB], bf16, tag="otp", bufs=4)
                nc.vector.tensor_mul(
                    otp[0:HD, :], ops[0:HD, :], rt[0:HD, QB:2 * QB])
                cdma = nc.gpsimd if nrm_q else nc.sync
                for (mt, lo, hi, dst) in pieces:
                    cdma.dma_start(
                        ot[lo:hi, mt, qb * QB:(qb + 1) * QB],
                        otp[dst:dst + hi - lo, :])

            # ================= wavefront rounds =================
            with tc.tile_pool(name="xt_pool", bufs=1) as xt_pool, \
                 tc.tile_pool(name="wv_pool", bufs=1) as wv_pool, \
                 tc.tile_pool(name="wqk_pool", bufs=4) as wqk_pool:
                xt = xt_pool.tile([128, N_KT, N], bf16)
                wv_sb = wv_pool.tile([128, N_KT, DIM], bf16)

                # minimal-dependency-first startup DMAs.  Priority order
                # round-robined over 4 engine queues: first the small
                # weights, then x's qb0 halves (first QK chains stream as
                # k-tiles land), wv's vb0 column chunk (round-0 v_chains
                # gate only on their own chunk), x qb1, wv vb1..3.
                wv_view = wv_d.rearrange("(kt kp) v -> kp kt v", kp=128)
                # x full k-tiles over sync+gpsimd (startup is chip-HBM
                # bound; singles land incrementally so QK chains stream as
                # tiles arrive); wv in vb column chunks on scalar so
                # v_chain(vb) gates only on its own chunk.
                w_tiles = {}
                for i, m in enumerate((0, 9)):
                    w_t = wqk_pool.tile([128, N_KT, 128], bf16, tag="w")
                    (nc.gpsimd if i == 0 else nc.scalar).dma_start(
                        w_t, wqk_d[m])
                    w_tiles[m] = w_t
                nc.scalar.dma_start(bqk_sb, bqk_d)
                for kt in range(N_KT):
                    (nc.sync if kt % 2 == 0 else nc.gpsimd).dma_start(
                        xt[:, kt, :], x_d[:, kt, :])
                for vb in range(N_VB):
                    nc.scalar.dma_start(
                        wv_sb[:, :, vb * VB:(vb + 1) * VB],
                        wv_view[:, :, vb * VB:(vb + 1) * VB])
                nc.gpsimd.dma_start(bv_sb[0:1, :], bv_d)
                nc.gpsimd.partition_broadcast(bv_sb, bv_sb[0:1, :],
                                              channels=128)
                nc.vector.memset(vpad[:, :, :, HD:HD + 1], 1.0)

                head_tiles = {}   # h -> (ktp, qtp)
                head_es = {}      # (h, qb) -> es_tiles

                for r in range(N_PAIR + 1):
                    batch = HEADS_BY_PAIR[r]
                    # -- QK projection pair r --
                    if r < N_PAIR:
                        for m in (r, 9 + r):
                            for qb in range(N_QB):
                                qk_chain(xt, w_tiles[m], m, qb)
                        # prefetch next pair's weights (gpsimd, not scalar:
                        # the scalar ring carries the slow strided wv
                        # chunks and would delay these)
                        if r + 1 < N_PAIR:
                            for i, m in enumerate((r + 1, 10 + r)):
                                w_t = wqk_pool.tile([128, N_KT, 128], bf16,
                                                    tag="w")
                                (nc.sync if i == 0 else nc.gpsimd).dma_start(
                                    w_t, wqk_d[m])
                                w_tiles[m] = w_t
                    if r == 5:
                        nc.gpsimd.dma_start(bproj_sb[0:1, :], bproj_d)
                        nc.gpsimd.partition_broadcast(
                            bproj_sb, bproj_sb[0:1, :], channels=128)
                    if r == 7:
                        # prefetch wproj k-tiles 0..2 so the projection
                        # phase doesn't stall on its weight DMAs
                        for i in range(3):
                            (nc.sync if i % 2 == 0 else nc.gpsimd).dma_start(
                                wp_early[:, i, :], wproj_d[:, i, :])

                    # -- S for this round's heads (repacked last round) --
                    sq = []  # (h, qb) in emission order
                    for i, h in enumerate(batch):
                        ktp, qtp = head_tiles[h]
                        for qb in range(N_QB):
                            if i == len(batch) - 1 and qb == 1:
                                break  # last S emitted after some V work
                            head_es[(h, qb)] = s_block(h, qb, ktp, qtp)
                            sq.append((h, qb))

                    # -- V chains + deferred last S --
                    vb, vts = V_BY_ROUND.get(r, (0, []))
                    for tt in vts[:2]:
                        v_chain(xt, wv_sb, vb, tt)
                    if batch:
                        h = batch[-1]
                        ktp, qtp = head_tiles[h]
                        head_es[(h, 1)] = s_block(h, 1, ktp, qtp)
                        sq.append((h, 1))
                    for tt in vts[2:]:
                        v_chain(xt, wv_sb, vb, tt)

                    # -- AV + normalize (exp has had a full V block) --
                    for i, (h, qb) in enumerate(sq):
                        av_block(h, qb, head_es.pop((h, qb)), i % 2)
                    for h in batch:
                        head_tiles.pop(h)

                    # -- repacks for next round's heads --
                    if r + 1 <= N_PAIR:
                        for h in HEADS_BY_PAIR[r + 1] if r + 1 < len(
                                HEADS_BY_PAIR) else []:
                            head_tiles[h] = issue_repacks(h)

            if debug_taps:
                with tc.tile_pool(name="dbg_pool", bufs=2) as dbg_pool:
                    nc.gpsimd.dma_start(qkt_d, qkt)
                    for tt in range(N_TT):
                        cv = dbg_pool.tile([128, HEADS, HD + 1], f32,
                                           tag="cv")
                        nc.vector.tensor_copy(cv, vpad[:, tt])
                        nc.sync.dma_start(vpad_d[:, tt], cv)
                    for hh in range(N_KT):
                        co = dbg_pool.tile([128, N], f32, tag="co")
                        nc.vector.tensor_copy(co, ot[:, hh])
                        nc.sync.dma_start(ot_d[:, hh], co)

            # ================= output projection =================
            # s/o PSUM pools freed first so the proj chains can rotate 4
            # PSUM banks (DVE bias-add reads fully off the critical path).
            o_stack.__exit__(None, None, None)
            s_stack.__exit__(None, None, None)
            with tc.tile_pool(name="mmp_ps", bufs=1, space="PSUM") as mmp_ps, \
                 tc.tile_pool(name="wp_pool", bufs=1) as wp_pool, \
                 tc.tile_pool(name="out_pool", bufs=3) as out_pool:
                wp_sb = wp_pool.tile([128, N_KT - 3, DIM], bf16)
                for kt in range(3, N_KT):
                    (nc.sync if kt % 2 == 0 else nc.gpsimd).dma_start(
                        wp_sb[:, kt - 3, :], wproj_d[:, kt, :])
                for tt in range(N_TT):
                    outs = out_pool.tile([128, DIM], f32, tag="out")
                    for eb in range(N_EB):
                        ps = mmp_ps.tile([128, QB], f32, tag="mmp", bufs=4)
                        for dt in range(N_KT):
                            wsrc = (wp_early[:, dt, eb * EB:(eb + 1) * EB]
                                    if dt < 3 else
                                    wp_sb[:, dt - 3, eb * EB:(eb + 1) * EB])
                            nc.tensor.matmul(
                                ps[:, 0:EB],
                                lhsT=ot[:, dt, tt * 128:(tt + 1) * 128],
                                rhs=wsrc,
                                start=(dt == 0), stop=(dt == N_KT - 1))
                        nc.vector.tensor_add(
                            outs[:, eb * EB:(eb + 1) * EB], ps[:, 0:EB],
                            bproj_sb[:, eb * EB:(eb + 1) * EB])
                        nc.sync.dma_start(
                            out_d[tt * 128:(tt + 1) * 128,
                                  eb * EB:(eb + 1) * EB],
                            outs[:, eb * EB:(eb + 1) * EB])

            mm_stack.__exit__(None, None, None)
            wp_early_stack.__exit__(None, None, None)
            r_stack.__exit__(None, None, None)
            es_stack.__exit__(None, None, None)
            pad_stack.__exit__(None, None, None)

    nc.compile()
    return nc


def _get_nc(debug_taps=False):
    key = ("nc", debug_taps)
    if key not in _CACHE:
        _CACHE[key] = _build(debug_taps)
    return _CACHE[key]


def _prep_shared(Wqkv, bqkv, Wproj, bproj):
    """Host-side pure-layout transforms of the (replicated) weights."""
    Wqkv = np.asarray(Wqkv, dtype=np.float32)
    bqkv = np.asarray(bqkv, dtype=np.float32)
    Wproj = np.asarray(Wproj, dtype=np.float32)
    bproj = np.asarray(bproj, dtype=np.float32)

    wqk = np.ascontiguousarray(
        Wqkv[:, :QKDIM].reshape(N_KT, 128, N_MT_QK, 128).transpose(2, 1, 0, 3)
    ).astype(ml_dtypes.bfloat16)
    wv = np.ascontiguousarray(Wqkv[:, QKDIM:]).astype(ml_dtypes.bfloat16)
    bqk = np.ascontiguousarray(bqkv[:QKDIM].reshape(N_MT_QK, 128).T)
    bv = np.ascontiguousarray(bqkv[QKDIM:].reshape(1, DIM))
    wproj = np.ascontiguousarray(
        Wproj.reshape(N_KT, 128, DIM).transpose(1, 0, 2)).astype(
            ml_dtypes.bfloat16)
    bproj2 = np.ascontiguousarray(bproj.reshape(1, DIM))
    return dict(wqk=wqk, wv=wv, bqk=bqk, bv=bv, wproj=wproj, bproj=bproj2)


def kernel(x, Wqkv, bqkv, Wproj, bproj, _trace=False, _debug_taps=False):
    from concourse import bass_utils

    x = np.asarray(x, dtype=np.float32).astype(ml_dtypes.bfloat16)
    # device layout: xt[p, dt, t] = x[t, dt*128+p]
    xt = x.transpose(0, 2, 1).reshape(NCORES, N_KT, 128, N).transpose(
        0, 2, 1, 3)
    shared = _prep_shared(Wqkv, bqkv, Wproj, bproj)
    in_maps = [dict(x=np.ascontiguousarray(xt[i]), **shared)
               for i in range(NCORES)]
    nc = _get_nc(_debug_taps)
    res = bass_utils.run_bass_kernel_spmd(
        nc, in_maps, core_ids=list(range(NCORES)), trace=_trace)
    out = np.stack([res.results[i]["out"] for i in range(NCORES)], axis=0)
    if _trace:
        _CACHE["last_exec_time_ns"] = res.exec_time_ns
        _CACHE["last_results"] = res
    return out



# revision 37
# speedup vs baseline: 1.0082x; 1.0082x over previous
"""Multi-head attention (B=8, N=1024, DIM=1152, H=16, hd=72) on 8 TRN2 cores.

Sharding: pure data parallelism -- core i computes batch element i, weights
are replicated. No collectives.

Wavefront schedule (per core): QK projection m-tiles are emitted in
(Q_j, K_j) PAIRS so head j's attention chain (repack -> S -> exp -> AV ->
normalize) runs one round behind the projection matmuls that produce its
rows.  This spreads the ~120us of ScalarE exp work over the whole kernel
instead of concentrating it in an attention phase where it outpaces the
PE (3.8us exp vs 3.5us matmul per head-block), and removes the cold-start
serialization (w/x DMAs ordered minimal-dependency-first across queues).

Perf notes from trace analysis (HW ~284-286us, PE-saturated):
  - PE stream floor is ~247.5us @2.4GHz; prologue+teardown barriers are
    ~16us fixed; startup x-feed is chip-HBM-bound (~6us idle).
  - S operands are padded to 96 contraction partitions with zero rows
    72..95 (<96-partition matmuls measured 2 cyc/row in isolation).
  - wproj k-tiles 0..2 are prefetched during round 7 into wp_early so
    the projection phase never stalls on weight DMAs; proj rotates 4
    PSUM banks (s/o pools freed first).
  - wv is loaded in per-vb column chunks so round-r v_chains gate only
    on their own chunk; wqk prefetches go on sync/gpsimd (scalar ring
    carries the slow strided wv transfers).
  - fp8 DoubleRow was measured at 1.0 cyc/output-row (2x contraction
    only); e4m3 rounding (~2.4%/operand) blows the 2e-2 budget on any
    single matmul, so everything stays bf16.

Numerics / layout notes (inherited from the phase-split version):
  - x arrives bf16 (host cast); x^T built by host relayout.
  - S^T = K_h @ Q_h^T puts softmax's k-reduction on PSUM partitions; the
    denominator is recovered free via a ones column appended to V.
  - exp on ScalarE with the 1/sqrt(hd) scale folded in; no max subtraction
    (scores are ~N(0,1)).
  - Normalization: denominator row stream_shuffled to quadrant 0,
    reciprocal_approx_fast, shuffled back, fused DVE multiply into bf16
    O^T, DMA-repacked into a compact [128, 9, N] stack for the projection.
"""

import sys

sys.path.insert(0, "/opt/trn_rl_repo")

import numpy as np
import ml_dtypes

B, N, DIM, HEADS = 8, 1024, 1152, 16
HD = DIM // HEADS  # 72
NCORES = 8
QKDIM = 2 * DIM  # 2304 (q and k outdims concatenated)
N_MT_QK = QKDIM // 128  # 18 m-tiles for Q,K
N_PAIR = 9  # (Q_j, K_j) m-tile pairs
N_KT = DIM // 128  # 9 contraction tiles
N_TT = N // 128  # 8 token tiles
QB = 512  # q block (moving dim) for S^T / qkv
N_QB = N // QB  # 2
VB = 288  # v block = 4 heads
N_VB = DIM // VB  # 4
EB = 384  # proj output block
N_EB = DIM // EB  # 3

# heads whose Q/K rows are fully covered once m-tile pairs 0..j are done
HEADS_BY_PAIR = [[], [0], [1, 2], [3, 4], [5, 6], [7], [8, 9], [10, 11],
                 [12, 13], [14, 15]]  # index = round r; heads from pair r-1
# (vb, token tiles) of V-projection chains per round; vb3's last two chains
# sit in round 8 so its exp burst has PE cover (AV h12/h13 needs vb3 done)
V_BY_ROUND = {0: (0, [0, 1, 2, 3]), 1: (0, [4, 5, 6, 7]),
              2: (1, [0, 1, 2, 3]), 3: (1, [4, 5, 6, 7]),
              4: (2, [0, 1, 2, 3]), 5: (2, [4, 5, 6, 7]),
              6: (3, [0, 1, 2, 3]), 7: (3, [4, 5]), 8: (3, [6, 7])}

_CACHE = {}


def _head_pieces(h):
    """Pieces covering rows [72h, 72h+72) of a 128-row-tiled stack, as
    (mtile, src_lo, src_hi, dst_lo)."""
    r0 = HD * h
    mt, p0 = divmod(r0, 128)
    ln = min(HD, 128 - p0)
    pieces = [(mt, p0, p0 + ln, 0)]
    if ln < HD:
        pieces.append((mt + 1, 0, HD - ln, ln))
    return pieces


def _build(debug_taps=False):
    import concourse.tile as tile
    from concourse import bacc, mybir

    f32 = mybir.dt.float32
    bf16 = mybir.dt.bfloat16
    Exp = mybir.ActivationFunctionType.Exp

    nc = bacc.Bacc("TRN2", target_bir_lowering=False, debug=False,
                   num_devices=NCORES)

    x_d = nc.dram_tensor("x", [128, N_KT, N], bf16,
                         kind="ExternalInput").ap()  # x^T, host-relayouted
    wqk_d = nc.dram_tensor("wqk", [N_MT_QK, 128, N_KT, 128], bf16,
                           kind="ExternalInput").ap()
    wv_d = nc.dram_tensor("wv", [DIM, DIM], bf16, kind="ExternalInput").ap()
    bqk_d = nc.dram_tensor("bqk", [128, N_MT_QK], f32,
                           kind="ExternalInput").ap()
    bv_d = nc.dram_tensor("bv", [1, DIM], f32, kind="ExternalInput").ap()
    wproj_d = nc.dram_tensor("wproj", [128, N_KT, DIM], bf16,
                             kind="ExternalInput").ap()
    bproj_d = nc.dram_tensor("bproj", [1, DIM], f32,
                             kind="ExternalInput").ap()
    out_d = nc.dram_tensor("out", [N, DIM], f32, kind="ExternalOutput").ap()
    if debug_taps:
        qkt_d = nc.dram_tensor("dbg_qkt", [128, N_MT_QK, N], f32,
                               kind="ExternalOutput").ap()
        vpad_d = nc.dram_tensor("dbg_vpad", [128, N_TT, HEADS, HD + 1], f32,
                                kind="ExternalOutput").ap()
        ot_d = nc.dram_tensor("dbg_ot", [128, N_KT, N], f32,
                              kind="ExternalOutput").ap()

    scale = float(HD) ** -0.5

    with tile.TileContext(nc) as tc:
        with tc.tile_pool(name="consts", bufs=1) as consts, \
             tc.tile_pool(name="persist", bufs=1) as persist:
            # ---- persistent activations ----
            qkt = persist.tile([128, N_MT_QK, N], bf16)   # Q^T,K^T stacked
            vpad = persist.tile([128, N_TT, HEADS, HD + 1], bf16)
            ot = persist.tile([128, N_KT, N], bf16)       # O^T compact stack

            bqk_sb = consts.tile([128, N_MT_QK], f32)
            bv_sb = consts.tile([128, DIM], f32)
            bproj_sb = consts.tile([128, DIM], f32)

            # pools that live for the whole schedule
            pad_stack = tc.tile_pool(name="qk_pad", bufs=1)
            pads = pad_stack.__enter__()
            es_stack = tc.tile_pool(name="es_pool", bufs=16)
            es_pool = es_stack.__enter__()
            r_stack = tc.tile_pool(name="r_pool", bufs=5)
            r_pool = r_stack.__enter__()
            wp_early_stack = tc.tile_pool(name="wp_early", bufs=1)
            wp_early_pool = wp_early_stack.__enter__()
            wp_early = wp_early_pool.tile([128, 3, DIM], bf16)
            mm_stack = tc.tile_pool(name="mm_ps", bufs=1, space="PSUM")
            mm_ps = mm_stack.__enter__()
            s_stack = tc.tile_pool(name="s_ps", bufs=2, space="PSUM")
            s_ps = s_stack.__enter__()
            o_stack = tc.tile_pool(name="o_ps", bufs=2, space="PSUM")
            o_ps = o_stack.__enter__()

            ident = list(range(32))

            def qk_chain(xt, w_t, m, qb):
                ps = mm_ps.tile([128, QB], f32, tag="mm", bufs=2)
                for kt in range(N_KT):
                    nc.tensor.matmul(
                        ps,
                        lhsT=w_t[:, kt, :],
                        rhs=xt[:, kt, qb * QB:(qb + 1) * QB],
                        start=(kt == 0), stop=(kt == N_KT - 1))
                nc.scalar.add(
                    qkt[:, m, qb * QB:(qb + 1) * QB], ps, bqk_sb[:, m:m + 1])

            def v_chain(xt, wv_sb, vb, tt):
                ps = mm_ps.tile([128, QB], f32, tag="mm", bufs=2)
                for kt in range(N_KT):
                    nc.tensor.matmul(
                        ps[:, 0:VB],
                        lhsT=xt[:, kt, tt * 128:(tt + 1) * 128],
                        rhs=wv_sb[:, kt, vb * VB:(vb + 1) * VB],
                        start=(kt == 0), stop=(kt == N_KT - 1))
                nc.vector.tensor_add(
                    vpad[:, tt, 4 * vb:4 * vb + 4, 0:HD],
                    ps[:, 0:VB].rearrange("p (g d) -> p g d", g=4),
                    bv_sb[:, vb * VB:(vb + 1) * VB].rearrange(
                        "p (g d) -> p g d", g=4))

            # Repack targets: persistent tiles rotated manually so the
            # zero rows 72..95 survive across rounds (matmuls with <96
            # contraction partitions run at 2 cycles/row on TRN2, so S
            # operands are padded to 96 with zeros; zeros on BOTH sides so
            # no stale-NaN x 0 = NaN).
            ktp_bufs = [pads.tile([128, N], bf16, tag=f"ktp{i}",
                                  name=f"ktp{i}") for i in range(4)]
            qtp_bufs = [pads.tile([128, N], bf16, tag=f"qtp{i}",
                                  name=f"qtp{i}") for i in range(4)]
            for i in range(4):
                # 32-aligned partition base; rows 64..71 are re-written by
                # every repack DMA, rows 72..95 stay zero forever.
                nc.vector.memset(ktp_bufs[i][64:96, :], 0.0)
                nc.vector.memset(qtp_bufs[i][64:96, :], 0.0)
            repack_ctr = [0]

            def issue_repacks(h):
                """SBUF->SBUF DMAs move head h's K^T/Q^T rows to partition 0
                (matmul operands must start at partition 0/32/64)."""
                pieces = _head_pieces(h)
                i = repack_ctr[0] % 4
                repack_ctr[0] += 1
                ktp = ktp_bufs[i]
                qtp = qtp_bufs[i]
                for (mt, lo, hi, dst) in pieces:
                    nc.sync.dma_start(ktp[dst:dst + hi - lo, :],
                                      qkt[lo:hi, 9 + mt, :])
                    nc.gpsimd.dma_start(qtp[dst:dst + hi - lo, :],
                                        qkt[lo:hi, mt, :])
                return ktp, qtp

            def s_block(h, qb, ktp, qtp):
                es_tiles = []
                for kp in range(N_TT // 2):
                    ps = s_ps.tile([128, 2, QB], f32, tag="s")
                    for j in range(2):
                        kt = 2 * kp + j
                        nc.tensor.matmul(
                            ps[:, j, :],
                            lhsT=ktp[0:96, kt * 128:(kt + 1) * 128],
                            rhs=qtp[0:96, qb * QB:(qb + 1) * QB],
                            start=True, stop=True)
                    es = es_pool.tile([128, 2, QB], bf16, tag="e")
                    nc.scalar.activation(es, ps, func=Exp, scale=scale)
                    es_tiles.append(es)
                return es_tiles

            def av_block(h, qb, es_tiles, nrm_q):
                pieces = _head_pieces(h)
                ops = o_ps.tile([128, QB], f32, tag="o")
                for kt in range(N_TT):
                    nc.tensor.matmul(
                        ops[0:HD + 1, :],
                        lhsT=vpad[:, kt, h, :],
                        rhs=es_tiles[kt // 2][:, kt % 2, :],
                        start=(kt == 0), stop=(kt == N_TT - 1))
                # denominator (psum row 72) -> reciprocal broadcast rows 0..71
                rt = r_pool.tile([96, 2 * QB], f32, tag="r")
                nc.vector.stream_shuffle(
                    rt[0:32, 0:QB], ops[64:96, :], mask=[8] * 32)
                nc.vector.reciprocal_approx_fast(
                    rt[0:32, QB:2 * QB], rt[0:32, 0:QB])
                nc.vector.stream_shuffle(
                    rt[32:64, QB:2 * QB], rt[0:32, QB:2 * QB], mask=ident)
                nc.vector.stream_shuffle(
                    rt[64:96, QB:2 * QB], rt[0:32, QB:2 * QB], mask=ident)
                otp = pads.tile([128, QB], bf16, tag="otp", bufs=4)
                nc.vector.tensor_mul(
                    otp[0:HD, :], ops[0:HD, :], rt[0:HD, QB:2 * QB])
                cdma = nc.gpsimd if nrm_q else nc.sync
                for (mt, lo, hi, dst) in pieces:
                    cdma.dma_start(
                        ot[lo:hi, mt, qb * QB:(qb + 1) * QB],
                        otp[dst:dst + hi - lo, :])

            # ================= wavefront rounds =================
            with tc.tile_pool(name="xt_pool", bufs=1) as xt_pool, \
                 tc.tile_pool(name="wv_pool", bufs=1) as wv_pool, \
                 tc.tile_pool(name="wqk_pool", bufs=4) as wqk_pool:
                xt = xt_pool.tile([128, N_KT, N], bf16)
                wv_sb = wv_pool.tile([128, N_KT, DIM], bf16)

                # minimal-dependency-first startup DMAs.  Priority order
                # round-robined over 4 engine queues: first the small
                # weights, then x's qb0 halves (first QK chains stream as
                # k-tiles land), wv's vb0 column chunk (round-0 v_chains
                # gate only on their own chunk), x qb1, wv vb1..3.
                wv_view = wv_d.rearrange("(kt kp) v -> kp kt v", kp=128)
                # x full k-tiles over sync+gpsimd (startup is chip-HBM
                # bound; singles land incrementally so QK chains stream as
                # tiles arrive); wv in vb column chunks on scalar so
                # v_chain(vb) gates only on its own chunk.
                w_tiles = {}
                for i, m in enumerate((0, 9)):
                    w_t = wqk_pool.tile([128, N_KT, 128], bf16, tag="w")
                    (nc.sync if i == 0 else nc.gpsimd).dma_start(w_t, wqk_d[m])
                    w_tiles[m] = w_t
                nc.scalar.dma_start(bqk_sb, bqk_d)
                for kt in range(N_KT):
                    (nc.sync if kt % 2 == 0 else nc.gpsimd).dma_start(
                        xt[:, kt, :], x_d[:, kt, :])
                for vb in range(N_VB):
                    nc.scalar.dma_start(
                        wv_sb[:, :, vb * VB:(vb + 1) * VB],
                        wv_view[:, :, vb * VB:(vb + 1) * VB])
                nc.gpsimd.dma_start(bv_sb[0:1, :], bv_d)
                nc.gpsimd.partition_broadcast(bv_sb, bv_sb[0:1, :],
                                              channels=128)
                nc.vector.memset(vpad[:, :, :, HD:HD + 1], 1.0)

                head_tiles = {}   # h -> (ktp, qtp)
                head_es = {}      # (h, qb) -> es_tiles

                for r in range(N_PAIR + 1):
                    batch = HEADS_BY_PAIR[r]
                    # -- QK projection pair r --
                    if r < N_PAIR:
                        for m in (r, 9 + r):
                            for qb in range(N_QB):
                                qk_chain(xt, w_tiles[m], m, qb)
                        # prefetch next pair's weights (gpsimd, not scalar:
                        # the scalar ring carries the slow strided wv
                        # chunks and would delay these)
                        if r + 1 < N_PAIR:
                            for i, m in enumerate((r + 1, 10 + r)):
                                w_t = wqk_pool.tile([128, N_KT, 128], bf16,
                                                    tag="w")
                                (nc.sync if i == 0 else nc.gpsimd).dma_start(
                                    w_t, wqk_d[m])
                                w_tiles[m] = w_t
                    if r == 5:
                        nc.gpsimd.dma_start(bproj_sb[0:1, :], bproj_d)
                        nc.gpsimd.partition_broadcast(
                            bproj_sb, bproj_sb[0:1, :], channels=128)
                    if r == 7:
                        # prefetch wproj k-tiles 0..2 so the projection
                        # phase doesn't stall on its weight DMAs
                        for i in range(3):
                            (nc.sync if i % 2 == 0 else nc.gpsimd).dma_start(
                                wp_early[:, i, :], wproj_d[:, i, :])

                    # -- S for this round's heads (repacked last round) --
                    sq = []  # (h, qb) in emission order
                    for i, h in enumerate(batch):
                        ktp, qtp = head_tiles[h]
                        for qb in range(N_QB):
                            if i == len(batch) - 1 and qb == 1:
                                break  # last S emitted after some V work
                            head_es[(h, qb)] = s_block(h, qb, ktp, qtp)
                            sq.append((h, qb))

                    # -- V chains + deferred last S --
                    vb, vts = V_BY_ROUND.get(r, (0, []))
                    for tt in vts[:2]:
                        v_chain(xt, wv_sb, vb, tt)
                    if batch:
                        h = batch[-1]
                        ktp, qtp = head_tiles[h]
                        head_es[(h, 1)] = s_block(h, 1, ktp, qtp)
                        sq.append((h, 1))
                    for tt in vts[2:]:
                        v_chain(xt, wv_sb, vb, tt)

                    # -- AV + normalize (exp has had a full V block) --
                    for i, (h, qb) in enumerate(sq):
                        av_block(h, qb, head_es.pop((h, qb)), i % 2)
                    for h in batch:
                        head_tiles.pop(h)

                    # -- repacks for next round's heads --
                    if r + 1 <= N_PAIR:
                        for h in HEADS_BY_PAIR[r + 1] if r + 1 < len(
                                HEADS_BY_PAIR) else []:
                            head_tiles[h] = issue_repacks(h)

            if debug_taps:
                with tc.tile_pool(name="dbg_pool", bufs=2) as dbg_pool:
                    nc.gpsimd.dma_start(qkt_d, qkt)
                    for tt in range(N_TT):
                        cv = dbg_pool.tile([128, HEADS, HD + 1], f32,
                                           tag="cv")
                        nc.vector.tensor_copy(cv, vpad[:, tt])
                        nc.sync.dma_start(vpad_d[:, tt], cv)
                    for hh in range(N_KT):
                        co = dbg_pool.tile([128, N], f32, tag="co")
                        nc.vector.tensor_copy(co, ot[:, hh])
                        nc.sync.dma_start(ot_d[:, hh], co)

            # ================= output projection =================
            # s/o PSUM pools freed first so the proj chains can rotate 4
            # PSUM banks (DVE bias-add reads fully off the critical path).
            o_stack.__exit__(None, None, None)
            s_stack.__exit__(None, None, None)
            with tc.tile_pool(name="mmp_ps", bufs=1, space="PSUM") as mmp_ps, \
                 tc.tile_pool(name="wp_pool", bufs=1) as wp_pool, \
                 tc.tile_pool(name="out_pool", bufs=3) as out_pool:
                wp_sb = wp_pool.tile([128, N_KT - 3, DIM], bf16)
                for kt in range(3, N_KT):
                    (nc.sync if kt % 2 == 0 else nc.gpsimd).dma_start(
                        wp_sb[:, kt - 3, :], wproj_d[:, kt, :])
                for tt in range(N_TT):
                    outs = out_pool.tile([128, DIM], f32, tag="out")
                    for eb in range(N_EB):
                        ps = mmp_ps.tile([128, QB], f32, tag="mmp", bufs=4)
                        for dt in range(N_KT):
                            wsrc = (wp_early[:, dt, eb * EB:(eb + 1) * EB]
                                    if dt < 3 else
                                    wp_sb[:, dt - 3, eb * EB:(eb + 1) * EB])
                            nc.tensor.matmul(
                                ps[:, 0:EB],
                                lhsT=ot[:, dt, tt * 128:(tt + 1) * 128],
                                rhs=wsrc,
                                start=(dt == 0), stop=(dt == N_KT - 1))
                        nc.vector.tensor_add(
                            outs[:, eb * EB:(eb + 1) * EB], ps[:, 0:EB],
                            bproj_sb[:, eb * EB:(eb + 1) * EB])
                        nc.sync.dma_start(
                            out_d[tt * 128:(tt + 1) * 128,
                                  eb * EB:(eb + 1) * EB],
                            outs[:, eb * EB:(eb + 1) * EB])

            mm_stack.__exit__(None, None, None)
            wp_early_stack.__exit__(None, None, None)
            r_stack.__exit__(None, None, None)
            es_stack.__exit__(None, None, None)
            pad_stack.__exit__(None, None, None)

    nc.compile()
    return nc


def _get_nc(debug_taps=False):
    key = ("nc", debug_taps)
    if key not in _CACHE:
        _CACHE[key] = _build(debug_taps)
    return _CACHE[key]


def _prep_shared(Wqkv, bqkv, Wproj, bproj):
    """Host-side pure-layout transforms of the (replicated) weights."""
    Wqkv = np.asarray(Wqkv, dtype=np.float32)
    bqkv = np.asarray(bqkv, dtype=np.float32)
    Wproj = np.asarray(Wproj, dtype=np.float32)
    bproj = np.asarray(bproj, dtype=np.float32)

    wqk = np.ascontiguousarray(
        Wqkv[:, :QKDIM].reshape(N_KT, 128, N_MT_QK, 128).transpose(2, 1, 0, 3)
    ).astype(ml_dtypes.bfloat16)
    wv = np.ascontiguousarray(Wqkv[:, QKDIM:]).astype(ml_dtypes.bfloat16)
    bqk = np.ascontiguousarray(bqkv[:QKDIM].reshape(N_MT_QK, 128).T)
    bv = np.ascontiguousarray(bqkv[QKDIM:].reshape(1, DIM))
    wproj = np.ascontiguousarray(
        Wproj.reshape(N_KT, 128, DIM).transpose(1, 0, 2)).astype(
            ml_dtypes.bfloat16)
    bproj2 = np.ascontiguousarray(bproj.reshape(1, DIM))
    return dict(wqk=wqk, wv=wv, bqk=bqk, bv=bv, wproj=wproj, bproj=bproj2)


def kernel(x, Wqkv, bqkv, Wproj, bproj, _trace=False, _debug_taps=False):
    from concourse import bass_utils

    x = np.asarray(x, dtype=np.float32).astype(ml_dtypes.bfloat16)
    # device layout: xt[p, dt, t] = x[t, dt*128+p]
    xt = x.transpose(0, 2, 1).reshape(NCORES, N_KT, 128, N).transpose(
        0, 2, 1, 3)
    shared = _prep_shared(Wqkv, bqkv, Wproj, bproj)
    in_maps = [dict(x=np.ascontiguousarray(xt[i]), **shared)
               for i in range(NCORES)]
    nc = _get_nc(_debug_taps)
    res = bass_utils.run_bass_kernel_spmd(
        nc, in_maps, core_ids=list(range(NCORES)), trace=_trace)
    out = np.stack([res.results[i]["out"] for i in range(NCORES)], axis=0)
    if _trace:
        _CACHE["last_exec_time_ns"] = res.exec_time_ns
        _CACHE["last_results"] = res
    return out

